# revision 14
# baseline (speedup 1.0000x reference)
"""BiAttention Trainium2 kernel v3 (8 NeuronCores, data-parallel over batch).

Per core: 2 batches. Per batch b:
  C_ = lrelu(C @ W1.T + b1); Q_ = lrelu(Q @ W1.T + b1)
  S  = C_ @ Q_.T, masked (Qmask cols -> -1e30), softmax over Lq
  A  = S_ @ Q
  cat = [C, A, A-C, A*C]
  fuse = tanh(cat @ Wf.T); gate = sigmoid(cat @ Wg.T)      [fp8 DoubleRow]
  out = gate*fuse + (1-gate)*cat

v3 changes over v2 (memory-regime rework):
  * Cmask row compaction on HOST: only unmasked C rows (~50%) are shipped
    to and processed on device (padded to a 128 multiple); masked rows are
    filled with -1e30 during the host-side scatter. All Cmask logic
    (cmc/bcc/cmch masking) is gone from the device kernel.
  * Qmask column compaction on HOST: only unmasked Q rows (~50%) survive
    (padded to a 128 multiple, padding masked via qma=-1e30), shrinking
    the softmax width Lq 512 -> ~384.
  * Activation dtypes slimmed for HBM traffic: C^T/Q/Q^T stream in as
    bf16 (PE truncates to fp22 anyway), the output streams out as fp16.
    C natural is derived on device by PE-transposing C^T; the fp8 C^T for
    the fused GEMM is quantized on device from bf16 C^T. Per-pass HBM
    traffic drops ~4.5x.
"""
import sys

sys.path.insert(0, "/opt/trn_rl_repo")

from collections import deque
from contextlib import ExitStack

import numpy as np
import ml_dtypes

import concourse.bass as bass
import concourse.tile as tile
from concourse import bacc, mybir
from concourse.bass_utils import run_bass_kernel_spmd
from concourse.masks import make_identity

F32 = mybir.dt.float32
F32R = mybir.dt.float32r
BF16 = mybir.dt.bfloat16
F16 = mybir.dt.float16
F8 = mybir.dt.float8e4
AF = mybir.ActivationFunctionType
ALU = mybir.AluOpType
AX = mybir.AxisListType
DR = mybir.MatmulPerfMode.DoubleRow
BF16_NP = ml_dtypes.bfloat16
F8_NP = ml_dtypes.float8_e4m3

N_CORES = 8
B_FULL, LC_FULL, LQ_FULL, D = 16, 2048, 512, 512
BPC = B_FULL // N_CORES

# fp8 scale plan: psum = 1024 * (cat @ weff.T); dequant in the activations.
SA_C, SA_A, SA_M = 8.0, 8.0, 2.0          # on-device activation scales
SW_C, SW_A, SW_M = 128.0, 128.0, 512.0    # host-side weight block scales
DEQ = 1.0 / 1024.0
CN_DMA = True   # C natural via DMA (True) vs PE transposes of C^T (False)


def _f32(ap):
    return ap.bitcast(F32)


def _body(ctx: ExitStack, tc, io, bpc, lc, lq, gemm_bias, repeat=1,
          w1bias=False):
    nc = tc.nc
    nct = lc // 128
    nqt = lq // 128
    # groups of up to 4 c-tiles (one ct/cq/cg staging block per group)
    groups = []
    s = 0
    while s < nct:
        groups.append((s, min(4, nct - s)))
        s += min(4, nct - s)
    ngrp = len(groups)
    gstart = [g[0] for g in groups]

    cons = ctx.enter_context(tc.tile_pool(name="cons", bufs=1))
    batch = ctx.enter_context(tc.tile_pool(name="batch", bufs=2))
    grp = ctx.enter_context(tc.tile_pool(name="grp", bufs=1))
    grp2 = ctx.enter_context(tc.tile_pool(name="grp2", bufs=2))
    cqp = ctx.enter_context(tc.tile_pool(name="cqp", bufs=2))
    chain = ctx.enter_context(tc.tile_pool(name="chain", bufs=2))
    psA = ctx.enter_context(tc.tile_pool(name="psA", bufs=2, space="PSUM"))
    psB = ctx.enter_context(tc.tile_pool(name="psB", bufs=2, space="PSUM"))
    psG = ctx.enter_context(tc.tile_pool(name="psG", bufs=4, space="PSUM"))

    # ---- weights: fp8 6 pair-tiles (2 k-slices x 2048 cols); DMAs
    # drip-fed at critical emission points (first batch only) ----
    wf_ch, wg_ch = [], []
    pending_w = []
    for jp in range(6):
        wf = cons.tile([128, 4096], F8, tag=f"wft{jp}")
        wg = cons.tile([128, 4096], F8, tag=f"wgt{jp}")
        pending_w.append((wf, io["wft"][:, jp * 4096:(jp + 1) * 4096]))
        pending_w.append((wg, io["wgt"][:, jp * 4096:(jp + 1) * 4096]))
        wf_ch.append(wf)
        wg_ch.append(wg)

    def drip_w(n):
        for _ in range(n):
            if pending_w:
                t, src = pending_w.pop(0)
                nc.sync.dma_start(t[:], src)

    w1t = cons.tile([128, 4 * D], BF16, tag="w1t")
    nc.sync.dma_start(w1t[:], io["w1t"])
    b1c = cons.tile([128, 4], F32, tag="b1c")
    nc.sync.dma_start(b1c[:], io["b1c"])
    ident_b = cons.tile([128, 128], BF16, tag="ident_b")
    if gemm_bias:
        bfg = cons.tile([64, 2048], BF16, tag="bfg")
        nc.sync.dma_start(bfg[0:1, :], io["bfr"])
        nc.sync.dma_start(bfg[32:33, :], io["bgr"])
        ones = cons.tile([64, 128], BF16, tag="ones")
        nc.vector.memset(ones[:], 1.0)
    with tc.tile_pool(name="init", bufs=1) as initp:
        ident = initp.tile([128, 128], F32, tag="ident")
        make_identity(nc, ident[:])
        nc.vector.tensor_copy(ident_b[:], ident[:])
    work = ctx.enter_context(tc.tile_pool(name="work", bufs=3))
    wout = ctx.enter_context(tc.tile_pool(name="wout", bufs=2))

    def qprep_dma(b, first):
        """qma + Q natural + Q^T DMA starts (all bf16)."""
        qma = batch.tile([128, lq], BF16, tag="qma")
        qb = batch.tile([128, nqt * D], BF16, tag="qb")
        q_t = batch.tile([128, 4 * lq], BF16, tag="q_t")
        qt = batch.tile([128, 4 * lq], BF16, tag="qt")
        for k in range(4):
            nc.sync.dma_start(qt[:, k * lq:(k + 1) * lq],
                              io["qt_in"][b, k * 128:(k + 1) * 128, :])
            if first:
                drip_w(1)
        for qi in range(nqt):
            nc.sync.dma_start(qb[:, qi * D:(qi + 1) * D],
                              io["q_in"][b, qi * 128:(qi + 1) * 128, :])
            if first and qi < 2:
                drip_w(1)
            if qi == 0:
                nc.sync.dma_start(qma[:], io["qma"][b])
        return {"qma": qma, "qb": qb, "q_t": q_t, "qt": qt}

    def qprep_mm(qc):
        """Q_^T = lrelu(W1 Q^T) matmuls."""
        qt = qc.pop("qt")
        for dc in range(4):
            ps = psA.tile([128, lq], F32, tag="psA")
            for k in range(4):
                nc.tensor.matmul(
                    ps[:], w1t[:, k * D + dc * 128:k * D + dc * 128 + 128],
                    qt[:, k * lq:(k + 1) * lq],
                    start=(k == 0), stop=(k == 3))
            nc.scalar.activation(
                qc["q_t"][:, dc * lq:(dc + 1) * lq], ps[:], AF.Prelu,
                bias=b1c[:, dc:dc + 1] if w1bias else 0.0,
                scale=1.0, alpha=0.01)

    def emit_ct_dma(b, g, drip=0):
        s0, ntg = groups[g]
        cols = ntg * 128
        ct = grp2.tile([128, 4 * 512], BF16, tag="ct_stage", name="ct_stage")
        for k in range(4):
            nc.sync.dma_start(ct[:, k * 512:k * 512 + cols],
                              io["ct_in"][b, k * 128:(k + 1) * 128,
                                          s0 * 128:s0 * 128 + cols])
            if drip:
                drip_w(drip)
        return ct

    def make_prologue_parts(b, g, ct):
        """4 closures: [cq quant half0, half1, C_^T dc01, C_^T dc23]."""
        s0, ntg = groups[g]
        cols = ntg * 128
        st = {"ct": ct}

        def cq_quant(half):
            def f():
                if "cq" not in st:
                    st["cq"] = cqp.tile([128, 4 * 512], F8, tag="cq_grp",
                                        name="cq_grp")
                nc.vector.tensor_scalar_mul(
                    st["cq"][:].rearrange("p (k c) -> p k c", k=4)[
                        :, 2 * half:2 * half + 2, :cols],
                    ct[:].rearrange("p (k c) -> p k c", k=4)[
                        :, 2 * half:2 * half + 2, :cols],
                    SA_C)
            return f

        def cmm(half):
            def f():
                if "cg" not in st:
                    st["cg"] = grp.tile([128, 4 * 512], BF16, tag="cgrp",
                                        name="cgrp")
                for dc in (2 * half, 2 * half + 1):
                    ps = psA.tile([128, 512], F32, tag="psA")
                    for k in range(4):
                        nc.tensor.matmul(
                            ps[:, :cols],
                            w1t[:, k * D + dc * 128:k * D + dc * 128 + 128],
                            ct[:, k * 512:k * 512 + cols],
                            start=(k == 0), stop=(k == 3))
                    nc.scalar.activation(
                        st["cg"][:, dc * 512:dc * 512 + cols], ps[:, :cols],
                        AF.Prelu,
                        bias=b1c[:, dc:dc + 1] if w1bias else 0.0,
                        scale=1.0, alpha=0.01)
            return f

        return [cq_quant(0), cq_quant(1), cmm(0), cmm(1)], st

    def make_chain_parts(b, g, t, pro_st, qc, st):
        ci = groups[g][0] + t

        def p0():  # S + q-masked softmax -> pb = exp(S - max) (bf16)
            ps_s = psA.tile([128, lq], F32, tag="psA")
            for dc in range(4):
                nc.tensor.matmul(
                    ps_s[:],
                    pro_st["cg"][:, dc * 512 + t * 128:dc * 512 + t * 128 + 128],
                    qc["q_t"][:, dc * lq:(dc + 1) * lq],
                    start=(dc == 0), stop=(dc == 3))
            s1 = chain.tile([128, lq], F32, tag="s1")
            nc.vector.tensor_add(s1[:], ps_s[:], qc["qma"][:])
            negm = chain.tile([128, 1], F32, tag="negm")
            nc.vector.reduce_max(negm[:], s1[:], axis=AX.X, negate=True)
            pb = chain.tile([128, lq], BF16, tag="pb")
            ssum = chain.tile([128, 1], F32, tag="ssum")
            nc.scalar.activation(pb[:], s1[:], AF.Exp, bias=negm[:],
                                 scale=1.0, accum_out=ssum[:])
            rec = chain.tile([128, 1], F32, tag="rec")
            nc.vector.reciprocal(rec[:], ssum[:])
            st["pb"] = pb
            st["rec"] = rec

        def p1():  # P^T (unnormalized) + C natural (DMA or ct transposes)
            ps_pt = psB.tile([128, 512], BF16, tag="psB")
            for qq in range(nqt):
                nc.tensor.transpose(ps_pt[:, qq * 128:(qq + 1) * 128],
                                    st["pb"][:, qq * 128:(qq + 1) * 128],
                                    ident_b[:])
            pt = chain.tile([128, nqt * 128], BF16, tag="pt")
            nc.vector.tensor_copy(pt[:], ps_pt[:, :nqt * 128])
            st["pt"] = pt
            c_nat = work.tile([128, D], BF16, tag="c_nat")
            if CN_DMA:
                nc.sync.dma_start(c_nat[:],
                                  io["c_in"][b, ci * 128:(ci + 1) * 128, :])
            else:
                ps_cn = psA.tile([128, 512], BF16, tag="psA")
                for k in range(4):
                    nc.tensor.transpose(
                        ps_cn[:, k * 128:(k + 1) * 128],
                        pro_st["ct"][:, k * 512 + t * 128:t * 128 + k * 512 + 128],
                        ident_b[:])
                nc.vector.tensor_copy(c_nat[:], ps_cn[:])
            st["c_nat"] = c_nat

        def p2():  # attn natural = (P^T).T @ Q / sum
            ps_an = psB.tile([128, 512], F32, tag="psB")
            for qq in range(nqt):
                nc.tensor.matmul(ps_an[:],
                                 st["pt"][:, qq * 128:(qq + 1) * 128],
                                 qc["qb"][:, qq * D:(qq + 1) * D],
                                 start=(qq == 0), stop=(qq == nqt - 1))
            attn = work.tile([128, 512], BF16, tag="attn")
            nc.vector.tensor_scalar(attn[:], ps_an[:], st["rec"][:], None,
                                    op0=ALU.mult)
            st["attn"] = attn

        def p3():  # attn^T + fp8 quantized catT blocks + A-C / A*C
            ps_b3 = psB.tile([128, 512], BF16, tag="psB")
            for dc in range(4):
                nc.tensor.transpose(ps_b3[:, dc * 128:(dc + 1) * 128],
                                    st["attn"][:, dc * 128:(dc + 1) * 128],
                                    ident_b[:])
            ct_sl = pro_st["ct"][:].rearrange("p (k c) -> p k c", k=4)[
                :, :, t * 128:(t + 1) * 128]
            attnq = work.tile([128, 512], F8, tag="attnq")
            nc.vector.tensor_scalar_mul(attnq[:], ps_b3[:], SA_A)
            multq = work.tile([128, 512], F8, tag="multq")
            nc.vector.scalar_tensor_tensor(
                multq[:].rearrange("p (k c) -> p k c", k=4),
                ps_b3[:].rearrange("p (k c) -> p k c", k=4), SA_M,
                ct_sl, op0=ALU.mult, op1=ALU.mult)
            st["cq"] = pro_st["cq"]
            amc = work.tile([128, 512], BF16, tag="amc")
            nc.gpsimd.tensor_sub(amc[:], st["attn"][:], st["c_nat"][:])
            amm = work.tile([128, 512], BF16, tag="amm")
            nc.vector.tensor_mul(amm[:], st["attn"][:], st["c_nat"][:])
            st.update(attnq=attnq, multq=multq, amc=amc, amm=amm)

        return [p0, p1, p2, p3]

    def gemm_stage(b, g, t, ch_st, qc, pend):
        ci = groups[g][0] + t
        key = (b, g, t)
        # everything this tile depends on must be emitted before its MMs
        while any(k == key for k, _ in pend):
            _, part = pend.popleft()
            part()
        emitted = 0
        cq_t = ch_st["cq"][:].rearrange("p (k c) -> p k c", k=4)[
            :, :, t * 128:(t + 1) * 128]
        # fc pairs share each stationary lhs across 4 accumulating MMs
        for half in range(2):
            while pend and emitted < 3 * half + 3:
                _, part = pend.popleft()
                part()
                emitted += 1
            fcs = (2 * half, 2 * half + 1)
            pss = [psG.tile([128, 512], F32, tag="psG", name=f"psG{i}")
                   for i in range(4)]  # [f_lo, g_lo, f_hi, g_hi]
            if gemm_bias:
                for i, fc in enumerate(fcs):
                    nc.tensor.matmul(pss[2 * i][:], ones[0:1, :],
                                     bfg[0:1, fc * 512:(fc + 1) * 512],
                                     start=True, stop=False)
                    nc.tensor.matmul(pss[2 * i + 1][:], ones[32:33, :],
                                     bfg[32:33, fc * 512:(fc + 1) * 512],
                                     start=True, stop=False)
            for jp in range(6):
                j2 = jp % 2
                if jp < 2:
                    lhs = cq_t[:, 2 * j2:2 * j2 + 2, :]
                else:
                    lhs = (ch_st["attnq"] if jp < 4 else
                           ch_st["multq"])[:].rearrange(
                        "p (k c) -> p k c", k=4)[:, 2 * j2:2 * j2 + 2, :]
                stt = (jp == 0) and not gemm_bias
                for i, fc in enumerate(fcs):
                    wf3 = wf_ch[jp][:].rearrange("p (k n) -> p k n", k=2)[
                        :, :, fc * 512:(fc + 1) * 512]
                    nc.tensor.matmul(pss[2 * i][:], lhs, wf3, start=stt,
                                     stop=(jp == 5), perf_mode=DR)
                    wg3 = wg_ch[jp][:].rearrange("p (k n) -> p k n", k=2)[
                        :, :, fc * 512:(fc + 1) * 512]
                    nc.tensor.matmul(pss[2 * i + 1][:], lhs, wg3, start=stt,
                                     stop=(jp == 5), perf_mode=DR)
            for i, fc in enumerate(fcs):
                fuse = chain.tile([128, 512], BF16, tag="fuse")
                nc.scalar.activation(fuse[:], pss[2 * i][:], AF.Tanh,
                                     scale=DEQ)
                gth = chain.tile([128, 512], BF16, tag="gth")
                nc.scalar.activation(gth[:], pss[2 * i + 1][:], AF.Tanh,
                                     scale=0.5 * DEQ)
                catm = [ch_st["c_nat"][:], ch_st["attn"][:],
                        ch_st["amc"][:], ch_st["amm"][:]][fc]
                # out = catm + 0.5*(gth+1)*(fuse-catm)
                d1 = chain.tile([128, 512], BF16, tag="d1")
                nc.gpsimd.tensor_sub(d1[:], fuse[:], catm)
                d2 = chain.tile([128, 512], BF16, tag="fuse")
                nc.vector.scalar_tensor_tensor(d2[:], gth[:], 1.0, d1[:],
                                               op0=ALU.add, op1=ALU.mult)
                if fc == 0:
                    ch_st["o16"] = wout.tile([128, 2048], F16, tag="out_t",
                                             name="o16")
                nc.vector.scalar_tensor_tensor(
                    ch_st["o16"][:, fc * 512:(fc + 1) * 512], d2[:],
                    0.5, catm, op0=ALU.mult, op1=ALU.add)
                if fc == 3:
                    nc.sync.dma_start(
                        io["out"][b, ci * 128:(ci + 1) * 128, :],
                        ch_st["o16"][:])

    seq = [bb for _ in range(repeat) for bb in range(bpc)]
    prefetched = {}
    for bi, b in enumerate(seq):
        if "qc" in prefetched:
            qc = prefetched.pop("qc")
            ct0 = prefetched.pop("ct0")
        else:
            qc = qprep_dma(b, first=(bi == 0))
            ct0 = emit_ct_dma(b, 0, drip=1 if bi == 0 else 0)
        qprep_mm(qc)
        pro_parts, pro_st0 = make_prologue_parts(b, 0, ct0)
        for p in pro_parts:
            p()
        ch_st0 = {}
        for p in make_chain_parts(b, 0, 0, pro_st0, qc, ch_st0):
            p()
        drip_w(12)

        pend = deque()
        pro_states = {0: pro_st0}
        ct_states = {}
        ch_states = {(0, 0): ch_st0}

        def queue_chain(gg, tt):
            stx = {}
            ch_states[(gg, tt)] = stx
            holder = {}

            def fmk(i):
                def f():
                    if "parts" not in holder:
                        holder["parts"] = make_chain_parts(
                            b, gg, tt, pro_states[gg], qc, stx)
                    holder["parts"][i]()
                return f
            for i in range(4):
                pend.append(((b, gg, tt), fmk(i)))

        def queue_prologue(gg):
            holder = {}

            def fmk(i):
                def f():
                    if "parts" not in holder:
                        holder["parts"], pst = make_prologue_parts(
                            b, gg, ct_states[gg])
                        pro_states[gg] = pst
                    holder["parts"][i]()
                return f
            for i in range(4):
                pend.append(((b, gg, 98), fmk(i)))

        def tile_of(idx):
            for gg in range(ngrp):
                s0, ntg = groups[gg]
                if idx < s0 + ntg:
                    return gg, idx - s0
            return None

        # lookahead: chain for tile idx+ahead queued while tile idx's GEMMs
        # are emitted; group prologue/dma queued before first use.
        ahead = 2
        for i in range(1, min(ahead, nct)):
            queue_chain(*tile_of(i))
        for g in range(ngrp):
            s0, ntg = groups[g]
            for t in range(ntg):
                idx = s0 + t
                if g + 1 < ngrp:
                    ns0 = gstart[g + 1]
                    if idx == max(s0, ns0 - 4):
                        def _ct(gg=g + 1):
                            def f():
                                ct_states[gg] = emit_ct_dma(b, gg)
                            return f
                        pend.append(((b, g, 99), _ct()))
                    if idx == max(s0, ns0 - 2):
                        queue_prologue(g + 1)
                nxt = idx + ahead
                if nxt < nct and tile_of(nxt) not in ch_states:
                    queue_chain(*tile_of(nxt))
                if idx == max(0, nct - 3) and bi + 1 < len(seq):
                    def _pref(bn=seq[bi + 1]):
                        def f():
                            prefetched["qc"] = qprep_dma(bn, False)
                            prefetched["ct0"] = emit_ct_dma(bn, 0)
                        return f
                    pend.append(((b, 97, 0), _pref()))

                gemm_stage(b, g, t, ch_states[(g, t)], qc, pend)
                ch_states.pop((g, t), None)
        while pend:
            pend.popleft()[1]()


_CACHE = {}


def _get_module(bpc, lc, lq, gemm_bias, repeat=1, w1bias=False):
    key = (bpc, lc, lq, gemm_bias, repeat, w1bias)
    if key in _CACHE:
        return _CACHE[key]
    nc = bacc.Bacc("TRN2", target_bir_lowering=False, debug=False,
                   num_devices=N_CORES)
    io = {
        "c_in": nc.dram_tensor("c_in", [bpc, lc, D], BF16,
                               kind="ExternalInput").ap(),
        "ct_in": nc.dram_tensor("ct_in", [bpc, D, lc], BF16,
                                kind="ExternalInput").ap(),
        "q_in": nc.dram_tensor("q_in", [bpc, lq, D], BF16,
                               kind="ExternalInput").ap(),
        "qt_in": nc.dram_tensor("qt_in", [bpc, D, lq], BF16,
                                kind="ExternalInput").ap(),
        "w1t": nc.dram_tensor("w1t", [128, 4 * D], BF16,
                              kind="ExternalInput").ap(),
        "wft": nc.dram_tensor("wft", [128, 12 * 2048], F8,
                              kind="ExternalInput").ap(),
        "wgt": nc.dram_tensor("wgt", [128, 12 * 2048], F8,
                              kind="ExternalInput").ap(),
        "b1c": nc.dram_tensor("b1c", [128, 4], F32, kind="ExternalInput").ap(),
        "bfr": nc.dram_tensor("bfr", [1, 2048], BF16,
                              kind="ExternalInput").ap(),
        "bgr": nc.dram_tensor("bgr", [1, 2048], BF16,
                              kind="ExternalInput").ap(),
        "qma": nc.dram_tensor("qma", [bpc, 128, lq], BF16,
                              kind="ExternalInput").ap(),
        "out": nc.dram_tensor("out", [bpc, lc, 4 * D], F16,
                              kind="ExternalOutput").ap(),
    }
    with tile.TileContext(nc) as tc, ExitStack() as ctx:
        _body(ctx, tc, io, bpc, lc, lq, gemm_bias, repeat, w1bias)
    nc.compile()
    _CACHE[key] = nc
    return nc


def _weff_fp8(W):
    b0, b1_, b2, b3 = (W[:, i * 512:(i + 1) * 512] for i in range(4))
    weff = np.concatenate(
        [(b0 - b2) * np.float32(SW_C), (b1_ + b2) * np.float32(SW_A),
         b3 * np.float32(SW_M)], axis=1)  # [2048, 1536]
    assert np.abs(weff).max() < 224.0, "fp8 weight overflow risk"
    wt = np.ascontiguousarray(weff.T)  # [1536, 2048]
    return np.ascontiguousarray(
        wt.reshape(12, 128, 2048).transpose(1, 0, 2).reshape(128, 12 * 2048)
    ).astype(F8_NP)


def _caps_from_masks(Cmask, Qmask):
    ccnt = np.asarray(Cmask).astype(np.int64).sum(axis=1)
    qcnt = np.asarray(Qmask).astype(np.int64).sum(axis=1)
    lc_cap = int(max(128, -(-int(ccnt.max()) // 128) * 128))
    lq_cap = int(max(128, -(-int(qcnt.max()) // 128) * 128))
    return lc_cap, lq_cap


def host_inputs(C, Q, Cmask, Qmask, W1, b1, Wf, bf, Wg, bg, bpc, lc, lq,
                n_cores):
    w1t = np.ascontiguousarray(
        np.ascontiguousarray(W1.T).reshape(4, 128, D)
        .transpose(1, 0, 2).reshape(128, 4 * D), dtype=np.float32
    ).astype(BF16_NP)
    wft, wgt = _weff_fp8(Wf), _weff_fp8(Wg)
    b1c = np.ascontiguousarray(b1.reshape(4, 128).T, dtype=np.float32)
    bsc = np.float32(1024.0)
    bfr = np.ascontiguousarray(bf.reshape(1, 2048) * bsc,
                               dtype=np.float32).astype(BF16_NP)
    bgr = np.ascontiguousarray(bg.reshape(1, 2048) * bsc,
                               dtype=np.float32).astype(BF16_NP)
    maps = []
    for i in range(n_cores):
        c_in = np.zeros((bpc, lc, D), dtype=BF16_NP)
        ct_in = np.zeros((bpc, D, lc), dtype=BF16_NP)
        q_in = np.zeros((bpc, lq, D), dtype=BF16_NP)
        qt_in = np.zeros((bpc, D, lq), dtype=BF16_NP)
        qma = np.zeros((bpc, 128, lq), dtype=np.float32)
        for bb in range(bpc):
            b = i * bpc + bb
            cidx = np.nonzero(Cmask[b])[0]
            qidx = np.nonzero(Qmask[b])[0]
            cc = np.asarray(C[b], dtype=np.float32)[cidx]     # [ncb, D]
            qq = np.asarray(Q[b], dtype=np.float32)[qidx]     # [nqb, D]
            ccb = cc.astype(BF16_NP)
            c_in[bb, :len(cidx), :] = ccb
            ct_in[bb, :, :len(cidx)] = ccb.T
            q_in[bb, :len(qidx), :] = qq.astype(BF16_NP)
            qt_in[bb, :, :len(qidx)] = qq.T.astype(BF16_NP)
            qma[bb, :, len(qidx):] = np.float32(-1e30)
        maps.append({
            "c_in": c_in,
            "ct_in": ct_in,
            "q_in": q_in,
            "qt_in": qt_in,
            "w1t": w1t, "wft": wft, "wgt": wgt, "b1c": b1c,
            "bfr": bfr, "bgr": bgr,
            "qma": qma.astype(BF16_NP),
        })
    return maps


def kernel(C, Q, Cmask, Qmask, W1, b1, Wf, bf, Wg, bg):
    C = np.asarray(C, dtype=np.float32)
    Q = np.asarray(Q, dtype=np.float32)
    Cmask = np.asarray(Cmask)
    Qmask = np.asarray(Qmask)
    W1 = np.asarray(W1, dtype=np.float32)
    b1 = np.asarray(b1, dtype=np.float32)
    Wf = np.asarray(Wf, dtype=np.float32)
    bf = np.asarray(bf, dtype=np.float32)
    Wg = np.asarray(Wg, dtype=np.float32)
    bg = np.asarray(bg, dtype=np.float32)

    gemm_bias = bool(np.any(bf) or np.any(bg))
    w1bias = bool(np.any(b1))
    lc_cap, lq_cap = _caps_from_masks(Cmask, Qmask)
    nc = _get_module(BPC, lc_cap, lq_cap, gemm_bias, w1bias=w1bias)
    maps = host_inputs(C, Q, Cmask, Qmask, W1, b1, Wf, bf, Wg, bg,
                       BPC, lc_cap, lq_cap, N_CORES)
    res = run_bass_kernel_spmd(nc, maps, list(range(N_CORES)))
    B, Lc = Cmask.shape
    out = np.full((B, Lc, 4 * D), np.float32(-1e30), dtype=np.float32)
    for i in range(N_CORES):
        for bb in range(BPC):
            b = i * BPC + bb
            cidx = np.nonzero(Cmask[b])[0]
            out[b, cidx, :] = res.results[i]["out"][bb][:len(cidx)].astype(
                np.float32)
    return out


# revision 20
# speedup vs baseline: 1.3421x; 1.3421x over previous
"""BiAttention Trainium2 kernel v3 (8 NeuronCores, data-parallel over batch).

Per core: 2 batches. Per batch b:
  C_ = lrelu(C @ W1.T + b1); Q_ = lrelu(Q @ W1.T + b1)
  S  = C_ @ Q_.T, masked (Qmask cols -> -1e30), softmax over Lq
  A  = S_ @ Q
  cat = [C, A, A-C, A*C]
  fuse = tanh(cat @ Wf.T); gate = sigmoid(cat @ Wg.T)      [fp8 DoubleRow]
  out = gate*fuse + (1-gate)*cat

v3 changes over v2 (memory-regime rework):
  * Cmask row compaction on HOST: only unmasked C rows (~50%) are shipped
    to and processed on device (padded to a 128 multiple); masked rows are
    filled with -1e30 during the host-side scatter. All Cmask logic
    (cmc/bcc/cmch masking) is gone from the device kernel.
  * Qmask column compaction on HOST: only unmasked Q rows (~50%) survive
    (padded to a 128 multiple, padding masked via qma=-1e30), shrinking
    the softmax width Lq 512 -> ~384.
  * Activation dtypes slimmed for HBM traffic: C^T/Q/Q^T stream in as
    bf16 (PE truncates to fp22 anyway), the output streams out as fp16.
    C natural is derived on device by PE-transposing C^T; the fp8 C^T for
    the fused GEMM is quantized on device from bf16 C^T. Per-pass HBM
    traffic drops ~4.5x.
"""
import sys

sys.path.insert(0, "/opt/trn_rl_repo")

from collections import deque
from contextlib import ExitStack

import numpy as np
import ml_dtypes

import concourse.bass as bass
import concourse.tile as tile
from concourse import bacc, mybir
from concourse.bass_utils import run_bass_kernel_spmd
from concourse.masks import make_identity

F32 = mybir.dt.float32
F32R = mybir.dt.float32r
BF16 = mybir.dt.bfloat16
F16 = mybir.dt.float16
F8 = mybir.dt.float8e4
AF = mybir.ActivationFunctionType
ALU = mybir.AluOpType
AX = mybir.AxisListType
DR = mybir.MatmulPerfMode.DoubleRow
BF16_NP = ml_dtypes.bfloat16
F8_NP = ml_dtypes.float8_e4m3

N_CORES = 8
B_FULL, LC_FULL, LQ_FULL, D = 16, 2048, 512, 512
BPC = B_FULL // N_CORES

# fp8 scale plan: psum = 1024 * (cat @ weff.T); dequant in the activations.
SA_C, SA_A, SA_M = 8.0, 8.0, 2.0          # on-device activation scales
SW_C, SW_A, SW_M = 128.0, 128.0, 512.0    # host-side weight block scales
DEQ = 1.0 / 1024.0
CN_DMA = True   # C natural via DMA (True) vs PE transposes of C^T (False)
AHEAD = 3       # chain lookahead depth (tiles in flight ahead of GEMM)


def _f32(ap):
    return ap.bitcast(F32)


def _body(ctx: ExitStack, tc, io, bpc, lc, lq, gemm_bias, repeat=1,
          w1bias=False):
    nc = tc.nc
    nct = lc // 128
    nqt = lq // 128
    # groups of up to 4 c-tiles (one ct/cq/cg staging block per group)
    groups = []
    s = 0
    while s < nct:
        groups.append((s, min(4, nct - s)))
        s += min(4, nct - s)
    ngrp = len(groups)
    gstart = [g[0] for g in groups]

    cons = ctx.enter_context(tc.tile_pool(name="cons", bufs=1))
    batch = ctx.enter_context(tc.tile_pool(name="batch", bufs=2))
    grp = ctx.enter_context(tc.tile_pool(name="grp", bufs=1))
    grp2 = ctx.enter_context(tc.tile_pool(name="grp2", bufs=2))
    cqp = ctx.enter_context(tc.tile_pool(name="cqp", bufs=2))
    chain = ctx.enter_context(tc.tile_pool(name="chain", bufs=AHEAD))
    psA = ctx.enter_context(tc.tile_pool(name="psA", bufs=2, space="PSUM"))
    psB = ctx.enter_context(tc.tile_pool(name="psB", bufs=2, space="PSUM"))
    psG = ctx.enter_context(tc.tile_pool(name="psG", bufs=4, space="PSUM"))

    # ---- weights: fp8 6 pair-tiles (2 k-slices x 2048 cols); DMAs
    # drip-fed at critical emission points (first batch only) ----
    wf_ch, wg_ch = [], []
    pending_w = []
    for jp in range(6):
        wf = cons.tile([128, 4096], F8, tag=f"wft{jp}")
        wg = cons.tile([128, 4096], F8, tag=f"wgt{jp}")
        pending_w.append((wf, io["wft"][:, jp * 4096:(jp + 1) * 4096]))
        pending_w.append((wg, io["wgt"][:, jp * 4096:(jp + 1) * 4096]))
        wf_ch.append(wf)
        wg_ch.append(wg)

    def drip_w(n):
        for _ in range(n):
            if pending_w:
                t, src = pending_w.pop(0)
                nc.sync.dma_start(t[:], src)

    w1t = cons.tile([128, 4 * D], BF16, tag="w1t")
    nc.sync.dma_start(w1t[:], io["w1t"])
    b1c = cons.tile([128, 4], F32, tag="b1c")
    nc.sync.dma_start(b1c[:], io["b1c"])
    ident_b = cons.tile([128, 128], BF16, tag="ident_b")
    if gemm_bias:
        bfg = cons.tile([64, 2048], BF16, tag="bfg")
        nc.sync.dma_start(bfg[0:1, :], io["bfr"])
        nc.sync.dma_start(bfg[32:33, :], io["bgr"])
        ones = cons.tile([64, 128], BF16, tag="ones")
        nc.vector.memset(ones[:], 1.0)
    with tc.tile_pool(name="init", bufs=1) as initp:
        ident = initp.tile([128, 128], F32, tag="ident")
        make_identity(nc, ident[:])
        nc.vector.tensor_copy(ident_b[:], ident[:])
    work = ctx.enter_context(tc.tile_pool(name="work", bufs=AHEAD + 1))
    wout = ctx.enter_context(tc.tile_pool(name="wout", bufs=2))

    def qprep_dma(b, first):
        """qma + Q natural + Q^T DMA starts (all bf16)."""
        qma = batch.tile([128, lq], BF16, tag="qma")
        qb = batch.tile([128, nqt * D], BF16, tag="qb")
        q_t = batch.tile([128, 4 * lq], BF16, tag="q_t")
        qt = batch.tile([128, 4 * lq], BF16, tag="qt")
        for k in range(4):
            nc.sync.dma_start(qt[:, k * lq:(k + 1) * lq],
                              io["qt_in"][b, k * 128:(k + 1) * 128, :])
            if first:
                drip_w(1)
        for qi in range(nqt):
            nc.sync.dma_start(qb[:, qi * D:(qi + 1) * D],
                              io["q_in"][b, qi * 128:(qi + 1) * 128, :])
            if first and qi < 2:
                drip_w(1)
            if qi == 0:
                nc.sync.dma_start(qma[:], io["qma"][b])
        return {"qma": qma, "qb": qb, "q_t": q_t, "qt": qt}

    def qprep_mm(qc):
        """Q_^T = lrelu(W1 Q^T) matmuls."""
        qt = qc.pop("qt")
        for dc in range(4):
            ps = psA.tile([128, lq], F32, tag="psA")
            for k in range(4):
                nc.tensor.matmul(
                    ps[:], w1t[:, k * D + dc * 128:k * D + dc * 128 + 128],
                    qt[:, k * lq:(k + 1) * lq],
                    start=(k == 0), stop=(k == 3))
            nc.scalar.activation(
                qc["q_t"][:, dc * lq:(dc + 1) * lq], ps[:], AF.Prelu,
                bias=b1c[:, dc:dc + 1] if w1bias else 0.0,
                scale=1.0, alpha=0.01)

    def emit_ct_dma(b, g, drip=0):
        s0, ntg = groups[g]
        cols = ntg * 128
        ct = grp2.tile([128, 4 * 512], BF16, tag="ct_stage", name="ct_stage")
        for k in range(4):
            nc.sync.dma_start(ct[:, k * 512:k * 512 + cols],
                              io["ct_in"][b, k * 128:(k + 1) * 128,
                                          s0 * 128:s0 * 128 + cols])
            if drip:
                drip_w(drip)
        return ct

    def make_prologue_parts(b, g, ct):
        """4 closures: [cq quant half0, half1, C_^T dc01, C_^T dc23]."""
        s0, ntg = groups[g]
        cols = ntg * 128
        st = {"ct": ct}

        def cq_quant(half):
            def f():
                if "cq" not in st:
                    st["cq"] = cqp.tile([128, 4 * 512], F8, tag="cq_grp",
                                        name="cq_grp")
                nc.vector.tensor_scalar_mul(
                    st["cq"][:].rearrange("p (k c) -> p k c", k=4)[
                        :, 2 * half:2 * half + 2, :cols],
                    ct[:].rearrange("p (k c) -> p k c", k=4)[
                        :, 2 * half:2 * half + 2, :cols],
                    SA_C)
            return f

        def cmm(half):
            def f():
                if "cg" not in st:
                    st["cg"] = grp.tile([128, 4 * 512], BF16, tag="cgrp",
                                        name="cgrp")
                for dc in (2 * half, 2 * half + 1):
                    ps = psA.tile([128, 512], F32, tag="psA")
                    for k in range(4):
                        nc.tensor.matmul(
                            ps[:, :cols],
                            w1t[:, k * D + dc * 128:k * D + dc * 128 + 128],
                            ct[:, k * 512:k * 512 + cols],
                            start=(k == 0), stop=(k == 3))
                    nc.scalar.activation(
                        st["cg"][:, dc * 512:dc * 512 + cols], ps[:, :cols],
                        AF.Prelu,
                        bias=b1c[:, dc:dc + 1] if w1bias else 0.0,
                        scale=1.0, alpha=0.01)
            return f

        return [cq_quant(0), cq_quant(1), cmm(0), cmm(1)], st

    def make_chain_parts(b, g, t, pro_st, qc, st):
        ci = groups[g][0] + t

        def p0():  # S + q-masked softmax -> pb = exp(S - max) (bf16)
            ps_s = psA.tile([128, lq], F32, tag="psA")
            for dc in range(4):
                nc.tensor.matmul(
                    ps_s[:],
                    pro_st["cg"][:, dc * 512 + t * 128:dc * 512 + t * 128 + 128],
                    qc["q_t"][:, dc * lq:(dc + 1) * lq],
                    start=(dc == 0), stop=(dc == 3))
            s1 = chain.tile([128, lq], F32, tag="s1")
            nc.vector.tensor_add(s1[:], ps_s[:], qc["qma"][:])
            negm = chain.tile([128, 1], F32, tag="negm")
            nc.vector.reduce_max(negm[:], s1[:], axis=AX.X, negate=True)
            pb = chain.tile([128, lq], BF16, tag="pb")
            ssum = chain.tile([128, 1], F32, tag="ssum")
            nc.scalar.activation(pb[:], s1[:], AF.Exp, bias=negm[:],
                                 scale=1.0, accum_out=ssum[:])
            rec = chain.tile([128, 1], F32, tag="rec")
            nc.vector.reciprocal(rec[:], ssum[:])
            st["pb"] = pb
            st["rec"] = rec

        def p1():  # P^T (unnormalized) + C natural (DMA or ct transposes)
            ps_pt = psB.tile([128, 512], BF16, tag="psB")
            for qq in range(nqt):
                nc.tensor.transpose(ps_pt[:, qq * 128:(qq + 1) * 128],
                                    st["pb"][:, qq * 128:(qq + 1) * 128],
                                    ident_b[:])
            pt = chain.tile([128, nqt * 128], BF16, tag="pt")
            nc.vector.tensor_copy(pt[:], ps_pt[:, :nqt * 128])
            st["pt"] = pt
            c_nat = work.tile([128, D], BF16, tag="c_nat")
            if CN_DMA:
                nc.sync.dma_start(c_nat[:],
                                  io["c_in"][b, ci * 128:(ci + 1) * 128, :])
            else:
                ps_cn = psA.tile([128, 512], BF16, tag="psA")
                for k in range(4):
                    nc.tensor.transpose(
                        ps_cn[:, k * 128:(k + 1) * 128],
                        pro_st["ct"][:, k * 512 + t * 128:t * 128 + k * 512 + 128],
                        ident_b[:])
                nc.vector.tensor_copy(c_nat[:], ps_cn[:])
            st["c_nat"] = c_nat

        def p2():  # attn natural = (P^T).T @ Q / sum
            ps_an = psB.tile([128, 512], F32, tag="psB")
            for qq in range(nqt):
                nc.tensor.matmul(ps_an[:],
                                 st["pt"][:, qq * 128:(qq + 1) * 128],
                                 qc["qb"][:, qq * D:(qq + 1) * D],
                                 start=(qq == 0), stop=(qq == nqt - 1))
            attn = work.tile([128, 512], BF16, tag="attn")
            nc.vector.tensor_scalar(attn[:], ps_an[:], st["rec"][:], None,
                                    op0=ALU.mult)
            st["attn"] = attn

        def p3():  # attn^T + fp8 quantized catT blocks + A-C / A*C
            ps_b3 = psB.tile([128, 512], BF16, tag="psB")
            for dc in range(4):
                nc.tensor.transpose(ps_b3[:, dc * 128:(dc + 1) * 128],
                                    st["attn"][:, dc * 128:(dc + 1) * 128],
                                    ident_b[:])
            ct_sl = pro_st["ct"][:].rearrange("p (k c) -> p k c", k=4)[
                :, :, t * 128:(t + 1) * 128]
            attnq = work.tile([128, 512], F8, tag="attnq")
            nc.vector.tensor_scalar_mul(attnq[:], ps_b3[:], SA_A)
            multq = work.tile([128, 512], F8, tag="multq")
            nc.vector.scalar_tensor_tensor(
                multq[:].rearrange("p (k c) -> p k c", k=4),
                ps_b3[:].rearrange("p (k c) -> p k c", k=4), SA_M,
                ct_sl, op0=ALU.mult, op1=ALU.mult)
            st["cq"] = pro_st["cq"]
            amc = work.tile([128, 512], BF16, tag="amc")
            nc.gpsimd.tensor_sub(amc[:], st["attn"][:], st["c_nat"][:])
            amm = work.tile([128, 512], BF16, tag="amm")
            nc.vector.tensor_mul(amm[:], st["attn"][:], st["c_nat"][:])
            st.update(attnq=attnq, multq=multq, amc=amc, amm=amm)

        return [p0, p1, p2, p3]

    def gemm_stage(b, g, t, ch_st, qc, pend):
        ci = groups[g][0] + t
        key = (b, g, t)
        # everything this tile depends on must be emitted before its MMs
        while any(k == key for k, _ in pend):
            _, part = pend.popleft()
            part()
        emitted = 0
        cq_t = ch_st["cq"][:].rearrange("p (k c) -> p k c", k=4)[
            :, :, t * 128:(t + 1) * 128]
        # fc pairs share each stationary lhs across 4 accumulating MMs
        for half in range(2):
            while pend and emitted < 3 * half + 3:
                _, part = pend.popleft()
                part()
                emitted += 1
            fcs = (2 * half, 2 * half + 1)
            pss = [psG.tile([128, 512], F32, tag="psG", name=f"psG{i}")
                   for i in range(4)]  # [f_lo, g_lo, f_hi, g_hi]
            if gemm_bias:
                for i, fc in enumerate(fcs):
                    nc.tensor.matmul(pss[2 * i][:], ones[0:1, :],
                                     bfg[0:1, fc * 512:(fc + 1) * 512],
                                     start=True, stop=False)
                    nc.tensor.matmul(pss[2 * i + 1][:], ones[32:33, :],
                                     bfg[32:33, fc * 512:(fc + 1) * 512],
                                     start=True, stop=False)
            for jp in range(6):
                j2 = jp % 2
                if jp < 2:
                    lhs = cq_t[:, 2 * j2:2 * j2 + 2, :]
                else:
                    lhs = (ch_st["attnq"] if jp < 4 else
                           ch_st["multq"])[:].rearrange(
                        "p (k c) -> p k c", k=4)[:, 2 * j2:2 * j2 + 2, :]
                stt = (jp == 0) and not gemm_bias
                for i, fc in enumerate(fcs):
                    wf3 = wf_ch[jp][:].rearrange("p (k n) -> p k n", k=2)[
                        :, :, fc * 512:(fc + 1) * 512]
                    nc.tensor.matmul(pss[2 * i][:], lhs, wf3, start=stt,
                                     stop=(jp == 5), perf_mode=DR)
                    wg3 = wg_ch[jp][:].rearrange("p (k n) -> p k n", k=2)[
                        :, :, fc * 512:(fc + 1) * 512]
                    nc.tensor.matmul(pss[2 * i + 1][:], lhs, wg3, start=stt,
                                     stop=(jp == 5), perf_mode=DR)
            for i, fc in enumerate(fcs):
                fuse = chain.tile([128, 512], BF16, tag="fuse")
                nc.scalar.activation(fuse[:], pss[2 * i][:], AF.Tanh,
                                     scale=DEQ)
                gth = chain.tile([128, 512], BF16, tag="gth")
                nc.scalar.activation(gth[:], pss[2 * i + 1][:], AF.Tanh,
                                     scale=0.5 * DEQ)
                catm = [ch_st["c_nat"][:], ch_st["attn"][:],
                        ch_st["amc"][:], ch_st["amm"][:]][fc]
                # out = catm + 0.5*(gth+1)*(fuse-catm)
                d1 = chain.tile([128, 512], BF16, tag="d1")
                nc.gpsimd.tensor_sub(d1[:], fuse[:], catm)
                d2 = chain.tile([128, 512], BF16, tag="fuse")
                nc.vector.scalar_tensor_tensor(d2[:], gth[:], 1.0, d1[:],
                                               op0=ALU.add, op1=ALU.mult)
                if fc == 0:
                    ch_st["o16"] = wout.tile([128, 2048], F16, tag="out_t",
                                             name="o16")
                nc.vector.scalar_tensor_tensor(
                    ch_st["o16"][:, fc * 512:(fc + 1) * 512], d2[:],
                    0.5, catm, op0=ALU.mult, op1=ALU.add)
                if fc == 3:
                    nc.sync.dma_start(
                        io["out"][b, ci * 128:(ci + 1) * 128, :],
                        ch_st["o16"][:])

    seq = [bb for _ in range(repeat) for bb in range(bpc)]
    prefetched = {}
    for bi, b in enumerate(seq):
        if "qc" in prefetched:
            qc = prefetched.pop("qc")
            ct0 = prefetched.pop("ct0")
        else:
            qc = qprep_dma(b, first=(bi == 0))
            ct0 = emit_ct_dma(b, 0, drip=1 if bi == 0 else 0)
        qprep_mm(qc)
        pro_parts, pro_st0 = make_prologue_parts(b, 0, ct0)
        for p in pro_parts:
            p()
        ch_st0 = {}
        for p in make_chain_parts(b, 0, 0, pro_st0, qc, ch_st0):
            p()
        drip_w(12)

        pend = deque()
        pro_states = {0: pro_st0}
        ct_states = {}
        ch_states = {(0, 0): ch_st0}

        def queue_chain(gg, tt):
            stx = {}
            ch_states[(gg, tt)] = stx
            holder = {}

            def fmk(i):
                def f():
                    if "parts" not in holder:
                        holder["parts"] = make_chain_parts(
                            b, gg, tt, pro_states[gg], qc, stx)
                    holder["parts"][i]()
                return f
            for i in range(4):
                pend.append(((b, gg, tt), fmk(i)))

        def queue_prologue(gg):
            holder = {}

            def fmk(i):
                def f():
                    if "parts" not in holder:
                        holder["parts"], pst = make_prologue_parts(
                            b, gg, ct_states[gg])
                        pro_states[gg] = pst
                    holder["parts"][i]()
                return f
            for i in range(4):
                pend.append(((b, gg, 98), fmk(i)))

        def tile_of(idx):
            for gg in range(ngrp):
                s0, ntg = groups[gg]
                if idx < s0 + ntg:
                    return gg, idx - s0
            return None

        # lookahead: chain for tile idx+ahead queued while tile idx's GEMMs
        # are emitted; group prologue/dma queued before first use.
        ahead = AHEAD
        for i in range(1, min(ahead, nct)):
            queue_chain(*tile_of(i))
        for g in range(ngrp):
            s0, ntg = groups[g]
            for t in range(ntg):
                idx = s0 + t
                if g + 1 < ngrp:
                    ns0 = gstart[g + 1]
                    if idx == max(s0, ns0 - 4):
                        def _ct(gg=g + 1):
                            def f():
                                ct_states[gg] = emit_ct_dma(b, gg)
                            return f
                        pend.append(((b, g, 99), _ct()))
                    if idx == max(s0, ns0 - ahead):
                        queue_prologue(g + 1)
                nxt = idx + ahead
                if nxt < nct and tile_of(nxt) not in ch_states:
                    queue_chain(*tile_of(nxt))
                if idx == max(0, nct - 3) and bi + 1 < len(seq):
                    def _pref(bn=seq[bi + 1]):
                        def f():
                            prefetched["qc"] = qprep_dma(bn, False)
                            prefetched["ct0"] = emit_ct_dma(bn, 0)
                        return f
                    pend.append(((b, 97, 0), _pref()))

                gemm_stage(b, g, t, ch_states[(g, t)], qc, pend)
                ch_states.pop((g, t), None)
        while pend:
            pend.popleft()[1]()


_CACHE = {}


def _get_module(bpc, lc, lq, gemm_bias, repeat=1, w1bias=False):
    key = (bpc, lc, lq, gemm_bias, repeat, w1bias)
    if key in _CACHE:
        return _CACHE[key]
    nc = bacc.Bacc("TRN2", target_bir_lowering=False, debug=False,
                   num_devices=N_CORES)
    io = {
        "c_in": nc.dram_tensor("c_in", [bpc, lc, D], BF16,
                               kind="ExternalInput").ap(),
        "ct_in": nc.dram_tensor("ct_in", [bpc, D, lc], BF16,
                                kind="ExternalInput").ap(),
        "q_in": nc.dram_tensor("q_in", [bpc, lq, D], BF16,
                               kind="ExternalInput").ap(),
        "qt_in": nc.dram_tensor("qt_in", [bpc, D, lq], BF16,
                                kind="ExternalInput").ap(),
        "w1t": nc.dram_tensor("w1t", [128, 4 * D], BF16,
                              kind="ExternalInput").ap(),
        "wft": nc.dram_tensor("wft", [128, 12 * 2048], F8,
                              kind="ExternalInput").ap(),
        "wgt": nc.dram_tensor("wgt", [128, 12 * 2048], F8,
                              kind="ExternalInput").ap(),
        "b1c": nc.dram_tensor("b1c", [128, 4], F32, kind="ExternalInput").ap(),
        "bfr": nc.dram_tensor("bfr", [1, 2048], BF16,
                              kind="ExternalInput").ap(),
        "bgr": nc.dram_tensor("bgr", [1, 2048], BF16,
                              kind="ExternalInput").ap(),
        "qma": nc.dram_tensor("qma", [bpc, 128, lq], BF16,
                              kind="ExternalInput").ap(),
        "out": nc.dram_tensor("out", [bpc, lc, 4 * D], F16,
                              kind="ExternalOutput").ap(),
    }
    with tile.TileContext(nc) as tc, ExitStack() as ctx:
        _body(ctx, tc, io, bpc, lc, lq, gemm_bias, repeat, w1bias)
    nc.compile()
    _CACHE[key] = nc
    return nc


def _weff_fp8(W):
    b0, b1_, b2, b3 = (W[:, i * 512:(i + 1) * 512] for i in range(4))
    weff = np.concatenate(
        [(b0 - b2) * np.float32(SW_C), (b1_ + b2) * np.float32(SW_A),
         b3 * np.float32(SW_M)], axis=1)  # [2048, 1536]
    assert np.abs(weff).max() < 224.0, "fp8 weight overflow risk"
    wt = np.ascontiguousarray(weff.T)  # [1536, 2048]
    return np.ascontiguousarray(
        wt.reshape(12, 128, 2048).transpose(1, 0, 2).reshape(128, 12 * 2048)
    ).astype(F8_NP)


def _caps_from_masks(Cmask, Qmask):
    ccnt = np.asarray(Cmask).astype(np.int64).sum(axis=1)
    qcnt = np.asarray(Qmask).astype(np.int64).sum(axis=1)
    lc_cap = int(max(128, -(-int(ccnt.max()) // 128) * 128))
    lq_cap = int(max(128, -(-int(qcnt.max()) // 128) * 128))
    return lc_cap, lq_cap


def host_inputs(C, Q, Cmask, Qmask, W1, b1, Wf, bf, Wg, bg, bpc, lc, lq,
                n_cores):
    w1t = np.ascontiguousarray(
        np.ascontiguousarray(W1.T).reshape(4, 128, D)
        .transpose(1, 0, 2).reshape(128, 4 * D), dtype=np.float32
    ).astype(BF16_NP)
    wft, wgt = _weff_fp8(Wf), _weff_fp8(Wg)
    b1c = np.ascontiguousarray(b1.reshape(4, 128).T, dtype=np.float32)
    bsc = np.float32(1024.0)
    bfr = np.ascontiguousarray(bf.reshape(1, 2048) * bsc,
                               dtype=np.float32).astype(BF16_NP)
    bgr = np.ascontiguousarray(bg.reshape(1, 2048) * bsc,
                               dtype=np.float32).astype(BF16_NP)
    maps = []
    for i in range(n_cores):
        c_in = np.zeros((bpc, lc, D), dtype=BF16_NP)
        ct_in = np.zeros((bpc, D, lc), dtype=BF16_NP)
        q_in = np.zeros((bpc, lq, D), dtype=BF16_NP)
        qt_in = np.zeros((bpc, D, lq), dtype=BF16_NP)
        qma = np.zeros((bpc, 128, lq), dtype=np.float32)
        for bb in range(bpc):
            b = i * bpc + bb
            cidx = np.nonzero(Cmask[b])[0]
            qidx = np.nonzero(Qmask[b])[0]
            cc = np.asarray(C[b], dtype=np.float32)[cidx]     # [ncb, D]
            qq = np.asarray(Q[b], dtype=np.float32)[qidx]     # [nqb, D]
            ccb = cc.astype(BF16_NP)
            c_in[bb, :len(cidx), :] = ccb
            ct_in[bb, :, :len(cidx)] = ccb.T
            q_in[bb, :len(qidx), :] = qq.astype(BF16_NP)
            qt_in[bb, :, :len(qidx)] = qq.T.astype(BF16_NP)
            qma[bb, :, len(qidx):] = np.float32(-1e30)
        maps.append({
            "c_in": c_in,
            "ct_in": ct_in,
            "q_in": q_in,
            "qt_in": qt_in,
            "w1t": w1t, "wft": wft, "wgt": wgt, "b1c": b1c,
            "bfr": bfr, "bgr": bgr,
            "qma": qma.astype(BF16_NP),
        })
    return maps


def kernel(C, Q, Cmask, Qmask, W1, b1, Wf, bf, Wg, bg):
    C = np.asarray(C, dtype=np.float32)
    Q = np.asarray(Q, dtype=np.float32)
    Cmask = np.asarray(Cmask)
    Qmask = np.asarray(Qmask)
    W1 = np.asarray(W1, dtype=np.float32)
    b1 = np.asarray(b1, dtype=np.float32)
    Wf = np.asarray(Wf, dtype=np.float32)
    bf = np.asarray(bf, dtype=np.float32)
    Wg = np.asarray(Wg, dtype=np.float32)
    bg = np.asarray(bg, dtype=np.float32)

    gemm_bias = bool(np.any(bf) or np.any(bg))
    w1bias = bool(np.any(b1))
    lc_cap, lq_cap = _caps_from_masks(Cmask, Qmask)
    nc = _get_module(BPC, lc_cap, lq_cap, gemm_bias, w1bias=w1bias)
    maps = host_inputs(C, Q, Cmask, Qmask, W1, b1, Wf, bf, Wg, bg,
                       BPC, lc_cap, lq_cap, N_CORES)
    res = run_bass_kernel_spmd(nc, maps, list(range(N_CORES)))
    B, Lc = Cmask.shape
    out = np.full((B, Lc, 4 * D), np.float32(-1e30), dtype=np.float32)
    for i in range(N_CORES):
        for bb in range(BPC):
            b = i * BPC + bb
            cidx = np.nonzero(Cmask[b])[0]
            out[b, cidx, :] = res.results[i]["out"][bb][:len(cidx)].astype(
                np.float32)
    return out


# revision 22
# speedup vs baseline: 1.3967x; 1.0406x over previous
"""BiAttention Trainium2 kernel v3 (8 NeuronCores, data-parallel over batch).

Per core: 2 batches. Per batch b:
  C_ = lrelu(C @ W1.T + b1); Q_ = lrelu(Q @ W1.T + b1)
  S  = C_ @ Q_.T, masked (Qmask cols -> -1e30), softmax over Lq
  A  = S_ @ Q
  cat = [C, A, A-C, A*C]
  fuse = tanh(cat @ Wf.T); gate = sigmoid(cat @ Wg.T)      [fp8 DoubleRow]
  out = gate*fuse + (1-gate)*cat

v3 changes over v2 (memory-regime rework):
  * Cmask row compaction on HOST: only unmasked C rows (~50%) are shipped
    to and processed on device (padded to a 128 multiple); masked rows are
    filled with -1e30 during the host-side scatter. All Cmask logic
    (cmc/bcc/cmch masking) is gone from the device kernel.
  * Qmask column compaction on HOST: only unmasked Q rows (~50%) survive
    (padded to a 128 multiple, padding masked via qma=-1e30), shrinking
    the softmax width Lq 512 -> ~384.
  * Activation dtypes slimmed for HBM traffic: C^T/Q/Q^T stream in as
    bf16 (PE truncates to fp22 anyway), the output streams out as fp16.
    C natural is derived on device by PE-transposing C^T; the fp8 C^T for
    the fused GEMM is quantized on device from bf16 C^T. Per-pass HBM
    traffic drops ~4.5x.
"""
import sys

sys.path.insert(0, "/opt/trn_rl_repo")

from collections import deque
from contextlib import ExitStack

import numpy as np
import ml_dtypes

import concourse.bass as bass
import concourse.tile as tile
from concourse import bacc, mybir
from concourse.bass_utils import run_bass_kernel_spmd
from concourse.masks import make_identity

F32 = mybir.dt.float32
F32R = mybir.dt.float32r
BF16 = mybir.dt.bfloat16
F16 = mybir.dt.float16
F8 = mybir.dt.float8e4
AF = mybir.ActivationFunctionType
ALU = mybir.AluOpType
AX = mybir.AxisListType
DR = mybir.MatmulPerfMode.DoubleRow
BF16_NP = ml_dtypes.bfloat16
F8_NP = ml_dtypes.float8_e4m3

N_CORES = 8
B_FULL, LC_FULL, LQ_FULL, D = 16, 2048, 512, 512
BPC = B_FULL // N_CORES

# fp8 scale plan: psum = 1024 * (cat @ weff.T); dequant in the activations.
SA_C, SA_A, SA_M = 8.0, 8.0, 2.0          # on-device activation scales
SW_C, SW_A, SW_M = 128.0, 128.0, 512.0    # host-side weight block scales
DEQ = 1.0 / 1024.0
CN_DMA = True   # C natural via DMA (True) vs PE transposes of C^T (False)
AHEAD = 3       # chain lookahead depth (tiles in flight ahead of GEMM)


def _f32(ap):
    return ap.bitcast(F32)


def _body(ctx: ExitStack, tc, io, bpc, lc, lq, gemm_bias, repeat=1,
          w1bias=False):
    nc = tc.nc
    nct = lc // 128
    nqt = lq // 128
    # groups of up to 4 c-tiles (one ct/cq/cg staging block per group)
    groups = []
    s = 0
    while s < nct:
        groups.append((s, min(4, nct - s)))
        s += min(4, nct - s)
    ngrp = len(groups)
    gstart = [g[0] for g in groups]

    cons = ctx.enter_context(tc.tile_pool(name="cons", bufs=1))
    batch = ctx.enter_context(tc.tile_pool(name="batch", bufs=2))
    grp = ctx.enter_context(tc.tile_pool(name="grp", bufs=1))
    grp2 = ctx.enter_context(tc.tile_pool(name="grp2", bufs=3))
    cqp = ctx.enter_context(tc.tile_pool(name="cqp", bufs=2))
    chain = ctx.enter_context(tc.tile_pool(name="chain", bufs=AHEAD))
    psA = ctx.enter_context(tc.tile_pool(name="psA", bufs=2, space="PSUM"))
    psB = ctx.enter_context(tc.tile_pool(name="psB", bufs=2, space="PSUM"))
    psG = ctx.enter_context(tc.tile_pool(name="psG", bufs=4, space="PSUM"))

    # ---- weights: fp8 6 pair-tiles (2 k-slices x 2048 cols); DMAs
    # drip-fed at critical emission points (first batch only) ----
    wf_ch, wg_ch = [], []
    pending_w = []
    for jp in range(6):
        wf = cons.tile([128, 4096], F8, tag=f"wft{jp}")
        wg = cons.tile([128, 4096], F8, tag=f"wgt{jp}")
        pending_w.append((wf, io["wft"][:, jp * 4096:(jp + 1) * 4096]))
        pending_w.append((wg, io["wgt"][:, jp * 4096:(jp + 1) * 4096]))
        wf_ch.append(wf)
        wg_ch.append(wg)

    def drip_w(n):
        for _ in range(n):
            if pending_w:
                t, src = pending_w.pop(0)
                nc.sync.dma_start(t[:], src)

    w1t = cons.tile([128, 4 * D], BF16, tag="w1t")
    nc.sync.dma_start(w1t[:], io["w1t"])
    b1c = cons.tile([128, 4], F32, tag="b1c")
    nc.sync.dma_start(b1c[:], io["b1c"])
    ident_b = cons.tile([128, 128], BF16, tag="ident_b")
    if gemm_bias:
        bfg = cons.tile([64, 2048], BF16, tag="bfg")
        nc.sync.dma_start(bfg[0:1, :], io["bfr"])
        nc.sync.dma_start(bfg[32:33, :], io["bgr"])
        ones = cons.tile([64, 128], BF16, tag="ones")
        nc.vector.memset(ones[:], 1.0)
    with tc.tile_pool(name="init", bufs=1) as initp:
        ident = initp.tile([128, 128], F32, tag="ident")
        make_identity(nc, ident[:])
        nc.vector.tensor_copy(ident_b[:], ident[:])
    work = ctx.enter_context(tc.tile_pool(name="work", bufs=AHEAD + 1))
    wout = ctx.enter_context(tc.tile_pool(name="wout", bufs=3))

    def qprep_dma(b, first):
        """qma + Q natural + Q^T DMA starts (all bf16)."""
        qma = batch.tile([128, lq], BF16, tag="qma")
        qb = batch.tile([128, nqt * D], BF16, tag="qb")
        q_t = batch.tile([128, 4 * lq], BF16, tag="q_t")
        qt = batch.tile([128, 4 * lq], BF16, tag="qt")
        for k in range(4):
            nc.sync.dma_start(qt[:, k * lq:(k + 1) * lq],
                              io["qt_in"][b, k * 128:(k + 1) * 128, :])
            if first:
                drip_w(1)
        for qi in range(nqt):
            nc.sync.dma_start(qb[:, qi * D:(qi + 1) * D],
                              io["q_in"][b, qi * 128:(qi + 1) * 128, :])
            if first and qi < 2:
                drip_w(1)
            if qi == 0:
                nc.sync.dma_start(qma[:], io["qma"][b])
        return {"qma": qma, "qb": qb, "q_t": q_t, "qt": qt}

    def qprep_mm(qc):
        """Q_^T = lrelu(W1 Q^T) matmuls."""
        qt = qc.pop("qt")
        for dc in range(4):
            ps = psA.tile([128, lq], F32, tag="psA")
            for k in range(4):
                nc.tensor.matmul(
                    ps[:], w1t[:, k * D + dc * 128:k * D + dc * 128 + 128],
                    qt[:, k * lq:(k + 1) * lq],
                    start=(k == 0), stop=(k == 3))
            nc.scalar.activation(
                qc["q_t"][:, dc * lq:(dc + 1) * lq], ps[:], AF.Prelu,
                bias=b1c[:, dc:dc + 1] if w1bias else 0.0,
                scale=1.0, alpha=0.01)

    def emit_ct_dma(b, g, drip=0):
        s0, ntg = groups[g]
        cols = ntg * 128
        ct = grp2.tile([128, 4 * 512], BF16, tag="ct_stage", name="ct_stage")
        for k in range(4):
            nc.sync.dma_start(ct[:, k * 512:k * 512 + cols],
                              io["ct_in"][b, k * 128:(k + 1) * 128,
                                          s0 * 128:s0 * 128 + cols])
            if drip:
                drip_w(drip)
        return ct

    def make_prologue_parts(b, g, ct):
        """4 closures: [cq quant half0, half1, C_^T dc01, C_^T dc23]."""
        s0, ntg = groups[g]
        cols = ntg * 128
        st = {"ct": ct}

        def cq_quant(half):
            def f():
                if "cq" not in st:
                    st["cq"] = cqp.tile([128, 4 * 512], F8, tag="cq_grp",
                                        name="cq_grp")
                nc.vector.tensor_scalar_mul(
                    st["cq"][:].rearrange("p (k c) -> p k c", k=4)[
                        :, 2 * half:2 * half + 2, :cols],
                    ct[:].rearrange("p (k c) -> p k c", k=4)[
                        :, 2 * half:2 * half + 2, :cols],
                    SA_C)
            return f

        def cmm(half):
            def f():
                if "cg" not in st:
                    st["cg"] = grp.tile([128, 4 * 512], BF16, tag="cgrp",
                                        name="cgrp")
                for dc in (2 * half, 2 * half + 1):
                    ps = psA.tile([128, 512], F32, tag="psA")
                    for k in range(4):
                        nc.tensor.matmul(
                            ps[:, :cols],
                            w1t[:, k * D + dc * 128:k * D + dc * 128 + 128],
                            ct[:, k * 512:k * 512 + cols],
                            start=(k == 0), stop=(k == 3))
                    nc.scalar.activation(
                        st["cg"][:, dc * 512:dc * 512 + cols], ps[:, :cols],
                        AF.Prelu,
                        bias=b1c[:, dc:dc + 1] if w1bias else 0.0,
                        scale=1.0, alpha=0.01)
            return f

        return [cq_quant(0), cq_quant(1), cmm(0), cmm(1)], st

    def make_chain_parts(b, g, t, pro_st, qc, st):
        ci = groups[g][0] + t

        def p0():  # S + q-masked softmax -> pb = exp(S - max) (bf16)
            ps_s = psA.tile([128, lq], F32, tag="psA")
            for dc in range(4):
                nc.tensor.matmul(
                    ps_s[:],
                    pro_st["cg"][:, dc * 512 + t * 128:dc * 512 + t * 128 + 128],
                    qc["q_t"][:, dc * lq:(dc + 1) * lq],
                    start=(dc == 0), stop=(dc == 3))
            s1 = chain.tile([128, lq], F32, tag="s1")
            nc.vector.tensor_add(s1[:], ps_s[:], qc["qma"][:])
            negm = chain.tile([128, 1], F32, tag="negm")
            nc.vector.reduce_max(negm[:], s1[:], axis=AX.X, negate=True)
            pb = chain.tile([128, lq], BF16, tag="pb")
            ssum = chain.tile([128, 1], F32, tag="ssum")
            nc.scalar.activation(pb[:], s1[:], AF.Exp, bias=negm[:],
                                 scale=1.0, accum_out=ssum[:])
            rec = chain.tile([128, 1], F32, tag="rec")
            nc.vector.reciprocal(rec[:], ssum[:])
            st["pb"] = pb
            st["rec"] = rec

        def p1():  # P^T (unnormalized) + C natural (DMA or ct transposes)
            ps_pt = psB.tile([128, 512], BF16, tag="psB")
            for qq in range(nqt):
                nc.tensor.transpose(ps_pt[:, qq * 128:(qq + 1) * 128],
                                    st["pb"][:, qq * 128:(qq + 1) * 128],
                                    ident_b[:])
            pt = chain.tile([128, nqt * 128], BF16, tag="pt")
            nc.vector.tensor_copy(pt[:], ps_pt[:, :nqt * 128])
            st["pt"] = pt
            c_nat = work.tile([128, D], BF16, tag="c_nat")
            if CN_DMA:
                nc.sync.dma_start(c_nat[:],
                                  io["c_in"][b, ci * 128:(ci + 1) * 128, :])
            else:
                ps_cn = psA.tile([128, 512], BF16, tag="psA")
                for k in range(4):
                    nc.tensor.transpose(
                        ps_cn[:, k * 128:(k + 1) * 128],
                        pro_st["ct"][:, k * 512 + t * 128:t * 128 + k * 512 + 128],
                        ident_b[:])
                nc.vector.tensor_copy(c_nat[:], ps_cn[:])
            st["c_nat"] = c_nat

        def p2():  # attn natural = (P^T).T @ Q / sum
            ps_an = psB.tile([128, 512], F32, tag="psB")
            for qq in range(nqt):
                nc.tensor.matmul(ps_an[:],
                                 st["pt"][:, qq * 128:(qq + 1) * 128],
                                 qc["qb"][:, qq * D:(qq + 1) * D],
                                 start=(qq == 0), stop=(qq == nqt - 1))
            attn = work.tile([128, 512], BF16, tag="attn")
            nc.vector.tensor_scalar(attn[:], ps_an[:], st["rec"][:], None,
                                    op0=ALU.mult)
            st["attn"] = attn

        def p3():  # attn^T + fp8 quantized catT blocks + A-C / A*C
            ps_b3 = psB.tile([128, 512], BF16, tag="psB")
            for dc in range(4):
                nc.tensor.transpose(ps_b3[:, dc * 128:(dc + 1) * 128],
                                    st["attn"][:, dc * 128:(dc + 1) * 128],
                                    ident_b[:])
            ct_sl = pro_st["ct"][:].rearrange("p (k c) -> p k c", k=4)[
                :, :, t * 128:(t + 1) * 128]
            attnq = work.tile([128, 512], F8, tag="attnq")
            nc.vector.tensor_scalar_mul(attnq[:], ps_b3[:], SA_A)
            multq = work.tile([128, 512], F8, tag="multq")
            nc.vector.scalar_tensor_tensor(
                multq[:].rearrange("p (k c) -> p k c", k=4),
                ps_b3[:].rearrange("p (k c) -> p k c", k=4), SA_M,
                ct_sl, op0=ALU.mult, op1=ALU.mult)
            st["cq"] = pro_st["cq"]
            amc = work.tile([128, 512], BF16, tag="amc")
            nc.gpsimd.tensor_sub(amc[:], st["attn"][:], st["c_nat"][:])
            amm = work.tile([128, 512], BF16, tag="amm")
            nc.vector.tensor_mul(amm[:], st["attn"][:], st["c_nat"][:])
            st.update(attnq=attnq, multq=multq, amc=amc, amm=amm)

        return [p0, p1, p2, p3]

    def gemm_stage(b, g, t, ch_st, qc, pend):
        ci = groups[g][0] + t
        key = (b, g, t)
        # everything this tile depends on must be emitted before its MMs
        while any(k == key for k, _ in pend):
            _, part = pend.popleft()
            part()
        emitted = 0
        cq_t = ch_st["cq"][:].rearrange("p (k c) -> p k c", k=4)[
            :, :, t * 128:(t + 1) * 128]
        # fc pairs share each stationary lhs across 4 accumulating MMs
        for half in range(2):
            while pend and emitted < 3 * half + 3:
                _, part = pend.popleft()
                part()
                emitted += 1
            fcs = (2 * half, 2 * half + 1)
            pss = [psG.tile([128, 512], F32, tag="psG", name=f"psG{i}")
                   for i in range(4)]  # [f_lo, g_lo, f_hi, g_hi]
            if gemm_bias:
                for i, fc in enumerate(fcs):
                    nc.tensor.matmul(pss[2 * i][:], ones[0:1, :],
                                     bfg[0:1, fc * 512:(fc + 1) * 512],
                                     start=True, stop=False)
                    nc.tensor.matmul(pss[2 * i + 1][:], ones[32:33, :],
                                     bfg[32:33, fc * 512:(fc + 1) * 512],
                                     start=True, stop=False)
            for jp in range(6):
                j2 = jp % 2
                if jp < 2:
                    lhs = cq_t[:, 2 * j2:2 * j2 + 2, :]
                else:
                    lhs = (ch_st["attnq"] if jp < 4 else
                           ch_st["multq"])[:].rearrange(
                        "p (k c) -> p k c", k=4)[:, 2 * j2:2 * j2 + 2, :]
                stt = (jp == 0) and not gemm_bias
                for i, fc in enumerate(fcs):
                    wf3 = wf_ch[jp][:].rearrange("p (k n) -> p k n", k=2)[
                        :, :, fc * 512:(fc + 1) * 512]
                    nc.tensor.matmul(pss[2 * i][:], lhs, wf3, start=stt,
                                     stop=(jp == 5), perf_mode=DR)
                    wg3 = wg_ch[jp][:].rearrange("p (k n) -> p k n", k=2)[
                        :, :, fc * 512:(fc + 1) * 512]
                    nc.tensor.matmul(pss[2 * i + 1][:], lhs, wg3, start=stt,
                                     stop=(jp == 5), perf_mode=DR)
            for i, fc in enumerate(fcs):
                fuse = chain.tile([128, 512], BF16, tag="fuse")
                nc.scalar.activation(fuse[:], pss[2 * i][:], AF.Tanh,
                                     scale=DEQ)
                gth = chain.tile([128, 512], BF16, tag="gth")
                nc.scalar.activation(gth[:], pss[2 * i + 1][:], AF.Tanh,
                                     scale=0.5 * DEQ)
                catm = [ch_st["c_nat"][:], ch_st["attn"][:],
                        ch_st["amc"][:], ch_st["amm"][:]][fc]
                # out = catm + 0.5*(gth+1)*(fuse-catm)
                d1 = chain.tile([128, 512], BF16, tag="d1")
                nc.gpsimd.tensor_sub(d1[:], fuse[:], catm)
                d2 = chain.tile([128, 512], BF16, tag="fuse")
                nc.vector.scalar_tensor_tensor(d2[:], gth[:], 1.0, d1[:],
                                               op0=ALU.add, op1=ALU.mult)
                if fc == 0:
                    ch_st["o16"] = wout.tile([128, 2048], F16, tag="out_t",
                                             name="o16")
                nc.vector.scalar_tensor_tensor(
                    ch_st["o16"][:, fc * 512:(fc + 1) * 512], d2[:],
                    0.5, catm, op0=ALU.mult, op1=ALU.add)
                if fc == 3:
                    nc.sync.dma_start(
                        io["out"][b, ci * 128:(ci + 1) * 128, :],
                        ch_st["o16"][:])

    seq = [bb for _ in range(repeat) for bb in range(bpc)]
    prefetched = {}
    for bi, b in enumerate(seq):
        if "qc" in prefetched:
            qc = prefetched.pop("qc")
            ct0 = prefetched.pop("ct0")
        else:
            qc = qprep_dma(b, first=(bi == 0))
            ct0 = emit_ct_dma(b, 0, drip=1 if bi == 0 else 0)
        qprep_mm(qc)
        pro_parts, pro_st0 = make_prologue_parts(b, 0, ct0)
        for p in pro_parts:
            p()
        ch_st0 = {}
        for p in make_chain_parts(b, 0, 0, pro_st0, qc, ch_st0):
            p()
        drip_w(12)

        pend = deque()
        pro_states = {0: pro_st0}
        ct_states = {}
        ch_states = {(0, 0): ch_st0}

        def queue_chain(gg, tt):
            stx = {}
            ch_states[(gg, tt)] = stx
            holder = {}

            def fmk(i):
                def f():
                    if "parts" not in holder:
                        holder["parts"] = make_chain_parts(
                            b, gg, tt, pro_states[gg], qc, stx)
                    holder["parts"][i]()
                return f
            for i in range(4):
                pend.append(((b, gg, tt), fmk(i)))

        def queue_prologue(gg):
            holder = {}

            def fmk(i):
                def f():
                    if "parts" not in holder:
                        holder["parts"], pst = make_prologue_parts(
                            b, gg, ct_states[gg])
                        pro_states[gg] = pst
                    holder["parts"][i]()
                return f
            for i in range(4):
                pend.append(((b, gg, 98), fmk(i)))

        def tile_of(idx):
            for gg in range(ngrp):
                s0, ntg = groups[gg]
                if idx < s0 + ntg:
                    return gg, idx - s0
            return None

        # lookahead: chain for tile idx+ahead queued while tile idx's GEMMs
        # are emitted; group prologue/dma queued before first use.
        ahead = AHEAD
        for i in range(1, min(ahead, nct)):
            queue_chain(*tile_of(i))
        for g in range(ngrp):
            s0, ntg = groups[g]
            for t in range(ntg):
                idx = s0 + t
                if g + 1 < ngrp:
                    ns0 = gstart[g + 1]
                    if idx == max(s0, ns0 - 4):
                        def _ct(gg=g + 1):
                            def f():
                                ct_states[gg] = emit_ct_dma(b, gg)
                            return f
                        pend.append(((b, g, 99), _ct()))
                    if idx == max(s0, ns0 - ahead):
                        queue_prologue(g + 1)
                nxt = idx + ahead
                if nxt < nct and tile_of(nxt) not in ch_states:
                    queue_chain(*tile_of(nxt))
                if idx == max(0, nct - 3) and bi + 1 < len(seq):
                    def _pref(bn=seq[bi + 1]):
                        def f():
                            prefetched["qc"] = qprep_dma(bn, False)
                            prefetched["ct0"] = emit_ct_dma(bn, 0)
                        return f
                    pend.append(((b, 97, 0), _pref()))

                gemm_stage(b, g, t, ch_states[(g, t)], qc, pend)
                ch_states.pop((g, t), None)
        while pend:
            pend.popleft()[1]()


_CACHE = {}


def _get_module(bpc, lc, lq, gemm_bias, repeat=1, w1bias=False):
    key = (bpc, lc, lq, gemm_bias, repeat, w1bias)
    if key in _CACHE:
        return _CACHE[key]
    nc = bacc.Bacc("TRN2", target_bir_lowering=False, debug=False,
                   num_devices=N_CORES)
    io = {
        "c_in": nc.dram_tensor("c_in", [bpc, lc, D], BF16,
                               kind="ExternalInput").ap(),
        "ct_in": nc.dram_tensor("ct_in", [bpc, D, lc], BF16,
                                kind="ExternalInput").ap(),
        "q_in": nc.dram_tensor("q_in", [bpc, lq, D], BF16,
                               kind="ExternalInput").ap(),
        "qt_in": nc.dram_tensor("qt_in", [bpc, D, lq], BF16,
                                kind="ExternalInput").ap(),
        "w1t": nc.dram_tensor("w1t", [128, 4 * D], BF16,
                              kind="ExternalInput").ap(),
        "wft": nc.dram_tensor("wft", [128, 12 * 2048], F8,
                              kind="ExternalInput").ap(),
        "wgt": nc.dram_tensor("wgt", [128, 12 * 2048], F8,
                              kind="ExternalInput").ap(),
        "b1c": nc.dram_tensor("b1c", [128, 4], F32, kind="ExternalInput").ap(),
        "bfr": nc.dram_tensor("bfr", [1, 2048], BF16,
                              kind="ExternalInput").ap(),
        "bgr": nc.dram_tensor("bgr", [1, 2048], BF16,
                              kind="ExternalInput").ap(),
        "qma": nc.dram_tensor("qma", [bpc, 128, lq], BF16,
                              kind="ExternalInput").ap(),
        "out": nc.dram_tensor("out", [bpc, lc, 4 * D], F16,
                              kind="ExternalOutput").ap(),
    }
    with tile.TileContext(nc) as tc, ExitStack() as ctx:
        _body(ctx, tc, io, bpc, lc, lq, gemm_bias, repeat, w1bias)
    nc.compile()
    _CACHE[key] = nc
    return nc


def _weff_fp8(W):
    b0, b1_, b2, b3 = (W[:, i * 512:(i + 1) * 512] for i in range(4))
    weff = np.concatenate(
        [(b0 - b2) * np.float32(SW_C), (b1_ + b2) * np.float32(SW_A),
         b3 * np.float32(SW_M)], axis=1)  # [2048, 1536]
    assert np.abs(weff).max() < 224.0, "fp8 weight overflow risk"
    wt = np.ascontiguousarray(weff.T)  # [1536, 2048]
    return np.ascontiguousarray(
        wt.reshape(12, 128, 2048).transpose(1, 0, 2).reshape(128, 12 * 2048)
    ).astype(F8_NP)


def _caps_from_masks(Cmask, Qmask):
    ccnt = np.asarray(Cmask).astype(np.int64).sum(axis=1)
    qcnt = np.asarray(Qmask).astype(np.int64).sum(axis=1)
    lc_cap = int(max(128, -(-int(ccnt.max()) // 128) * 128))
    lq_cap = int(max(128, -(-int(qcnt.max()) // 128) * 128))
    return lc_cap, lq_cap


def host_inputs(C, Q, Cmask, Qmask, W1, b1, Wf, bf, Wg, bg, bpc, lc, lq,
                n_cores):
    w1t = np.ascontiguousarray(
        np.ascontiguousarray(W1.T).reshape(4, 128, D)
        .transpose(1, 0, 2).reshape(128, 4 * D), dtype=np.float32
    ).astype(BF16_NP)
    wft, wgt = _weff_fp8(Wf), _weff_fp8(Wg)
    b1c = np.ascontiguousarray(b1.reshape(4, 128).T, dtype=np.float32)
    bsc = np.float32(1024.0)
    bfr = np.ascontiguousarray(bf.reshape(1, 2048) * bsc,
                               dtype=np.float32).astype(BF16_NP)
    bgr = np.ascontiguousarray(bg.reshape(1, 2048) * bsc,
                               dtype=np.float32).astype(BF16_NP)
    maps = []
    for i in range(n_cores):
        c_in = np.zeros((bpc, lc, D), dtype=BF16_NP)
        ct_in = np.zeros((bpc, D, lc), dtype=BF16_NP)
        q_in = np.zeros((bpc, lq, D), dtype=BF16_NP)
        qt_in = np.zeros((bpc, D, lq), dtype=BF16_NP)
        qma = np.zeros((bpc, 128, lq), dtype=np.float32)
        for bb in range(bpc):
            b = i * bpc + bb
            cidx = np.nonzero(Cmask[b])[0]
            qidx = np.nonzero(Qmask[b])[0]
            cc = np.asarray(C[b], dtype=np.float32)[cidx]     # [ncb, D]
            qq = np.asarray(Q[b], dtype=np.float32)[qidx]     # [nqb, D]
            ccb = cc.astype(BF16_NP)
            c_in[bb, :len(cidx), :] = ccb
            ct_in[bb, :, :len(cidx)] = ccb.T
            q_in[bb, :len(qidx), :] = qq.astype(BF16_NP)
            qt_in[bb, :, :len(qidx)] = qq.T.astype(BF16_NP)
            qma[bb, :, len(qidx):] = np.float32(-1e30)
        maps.append({
            "c_in": c_in,
            "ct_in": ct_in,
            "q_in": q_in,
            "qt_in": qt_in,
            "w1t": w1t, "wft": wft, "wgt": wgt, "b1c": b1c,
            "bfr": bfr, "bgr": bgr,
            "qma": qma.astype(BF16_NP),
        })
    return maps


def kernel(C, Q, Cmask, Qmask, W1, b1, Wf, bf, Wg, bg):
    C = np.asarray(C, dtype=np.float32)
    Q = np.asarray(Q, dtype=np.float32)
    Cmask = np.asarray(Cmask)
    Qmask = np.asarray(Qmask)
    W1 = np.asarray(W1, dtype=np.float32)
    b1 = np.asarray(b1, dtype=np.float32)
    Wf = np.asarray(Wf, dtype=np.float32)
    bf = np.asarray(bf, dtype=np.float32)
    Wg = np.asarray(Wg, dtype=np.float32)
    bg = np.asarray(bg, dtype=np.float32)

    gemm_bias = bool(np.any(bf) or np.any(bg))
    w1bias = bool(np.any(b1))
    lc_cap, lq_cap = _caps_from_masks(Cmask, Qmask)
    nc = _get_module(BPC, lc_cap, lq_cap, gemm_bias, w1bias=w1bias)
    maps = host_inputs(C, Q, Cmask, Qmask, W1, b1, Wf, bf, Wg, bg,
                       BPC, lc_cap, lq_cap, N_CORES)
    res = run_bass_kernel_spmd(nc, maps, list(range(N_CORES)))
    B, Lc = Cmask.shape
    out = np.full((B, Lc, 4 * D), np.float32(-1e30), dtype=np.float32)
    for i in range(N_CORES):
        for bb in range(BPC):
            b = i * BPC + bb
            cidx = np.nonzero(Cmask[b])[0]
            out[b, cidx, :] = res.results[i]["out"][bb][:len(cidx)].astype(
                np.float32)
    return out
